# revision 29
# baseline (speedup 1.0000x reference)
"""Trainium2 Bass kernel for the pairwise adjacency layer.

Reference math (B=1024 points, D=128 dims):
    a   = dc_param[0]
    e   = exp(1 - dc)                                  # [B, D]
    den[i,j] = mean_d((1-a)*(x[i]-x[j])**2 + a*e[i]*e[j])
    out = 1/den off-diagonal, 1.0 on the diagonal      # [B, B]

Expansion used on-chip (no [B,B,D] tensor ever materializes):
    den = c1*r_i + c1*r_j + c2*<x_i,x_j> + c3*<e_i,e_j>
    c1 = (1-a)/D, c2 = -2*c1, c3 = a/D, r_i = sum_d x[i,d]^2
so den is the sum of four K=128 matmuls accumulated in one PSUM bank:
    c2*G   = (c2*x_slab)^T @ x          (Gram term)
    c3*E   = (c3*e_slab)^T @ e          (exp cross term)
    c1*r_j = (c1*ones)^T   @ x2         (row broadcast of r)
    c1*r_i = (c1*x2_slab)^T @ ones      (column broadcast of r)
followed by one DVE reciprocal_approx_fast (~18 correct bits) and a
gpsimd affine_select stamping the diagonal to 1.0.

Sharding: pure output-row-parallel over 8 NeuronCores. Core c computes
output rows [c*128, (c+1)*128). Inputs x^T/dc^T are replicated to every
core (on-chip collectives bounce through HBM and have a ~10us/step
firmware floor - far worse than re-reading 1MB per core). Each core's
copy has its columns rotated left by c*128 so that the diagonal block of
the output is always local columns 0:128, making the diagonal stamp an
SPMD-uniform affine_select; the host unshard rolls each slab back.

Matmul operands are bf16 (full-rate PE streaming; fp32 PSUM
accumulation; rel err ~3e-3), everything else fp32. The runtime scalars
c1/c2/c3 are computed on ACT from dc_param and broadcast to all 128
partitions with a tiny rank-1 fp32 matmul. DMAs ride the SP HWDGE ring;
explicit add_dep_helper ordering edges keep the per-engine FIFOs free
of head-of-line blocking (DMA completion receipts are ~2.5-4us, so the
schedule is built around hiding them).
"""

import numpy as np

import concourse.bass as bass
import concourse.tile as tile
from concourse import bacc, mybir
from concourse.bass_utils import run_bass_kernel_spmd
from concourse.tile_rust import add_dep_helper

B = 1024          # number of points
D = 128           # feature dim
NCORES = 8
ROWS = B // NCORES  # output rows per core = 128
H = 512             # column half
F32 = mybir.dt.float32
BF16 = mybir.dt.bfloat16
AF = mybir.ActivationFunctionType


def build_nc():
    nc = bacc.Bacc(None, num_swdge_queues=2)
    xT = nc.declare_dram_parameter("xT", [D, B], F32, isOutput=False)
    dcT = nc.declare_dram_parameter("dcT", [D, B], F32, isOutput=False)
    apar = nc.declare_dram_parameter("apar", [1, 1], F32, isOutput=False)
    out = nc.declare_dram_parameter("out", [ROWS, B], F32, isOutput=True)

    with tile.TileContext(nc) as tc:
        with (
            tc.tile_pool(name="big", bufs=1) as big,
            tc.tile_pool(name="small", bufs=1) as small,
            tc.tile_pool(name="ps", bufs=1, space="PSUM") as ps,
        ):
            XB = [big.tile([D, H], BF16, name=f"XB{h}", tag=f"XB{h}") for h in range(2)]
            XT = [big.tile([D, H], F32, name=f"XT{h}", tag=f"XT{h}") for h in range(2)]
            DCT = [big.tile([D, H], F32, name=f"DCT{h}", tag=f"DCT{h}") for h in range(2)]
            ET = [big.tile([D, H], BF16, name=f"ET{h}", tag=f"ET{h}") for h in range(2)]
            X2 = [big.tile([D, H], BF16, name=f"X2{h}", tag=f"X2{h}") for h in range(2)]
            SIM = [big.tile([ROWS, H], F32, name=f"SIM{h}", tag=f"SIM{h}") for h in range(2)]
            XSC = small.tile([D, ROWS], BF16, tag="XSC")
            ESC = small.tile([D, ROWS], BF16, tag="ESC")
            X2SC = small.tile([D, ROWS], BF16, tag="X2SC")
            ONEM = small.tile([D, ROWS], BF16, tag="ONEM")
            ONESB = small.tile([D, H], BF16, tag="ONESB")
            ONESF = small.tile([1, D], F32, tag="ONESF")
            SA = small.tile([1, 1], F32, tag="SA")
            PK = small.tile([1, 4], F32, tag="PK")
            SB = small.tile([D, 4], F32, tag="SB")
            CONSTS = small.tile([D, 2], F32, tag="CONSTS")
            B1D = small.tile([1, 1], F32, tag="B1D")
            PS = [ps.tile([ROWS, H], F32, name=f"PS{h}", tag=f"PS{h}") for h in range(2)]
            PB = ps.tile([D, 4], F32, tag="PB")

            # ---- input DMAs ----
            # All on the SP HWDGE ring (keeps ACT free so its table load
            # starts immediately); interleave x/dc halves by first use.
            nc.sync.dma_start(SA[:], apar[:, :])
            nc.sync.dma_start(XT[0][:], xT[:, 0:H])
            nc.sync.dma_start(DCT[0][:], dcT[:, 0:H])
            nc.sync.dma_start(XT[1][:], xT[:, H:B])
            nc.sync.dma_start(DCT[1][:], dcT[:, H:B])

            # constants (DVE is idle while DMAs land)
            nc.vector.memset(ONESB[:], 1.0)
            nc.vector.memset(ONESF[:], 1.0)
            nc.vector.memset(CONSTS[:, 0:1], 0.0)
            nc.vector.memset(CONSTS[:, 1:2], 1.0)
            nc.vector.memset(B1D[:], 1.0 / D)
            ZB = CONSTS[:, 0:1]   # [128,1] zeros (bias operand)
            OB = CONSTS[:, 1:2]   # [128,1] ones  (bias operand)

            # ---- scalars from a = dc_param[0] (ACT) ----
            i_c1 = nc.scalar.activation(PK[0:1, 0:1], SA[0:1, 0:1],
                                        AF.Identity, bias=B1D[0:1, 0:1],
                                        scale=-1.0 / D)               # c1
            i_c2 = nc.scalar.activation(PK[0:1, 1:2], PK[0:1, 0:1], AF.Copy,
                                        scale=-2.0)                   # c2
            i_c3 = nc.scalar.activation(PK[0:1, 2:3], SA[0:1, 0:1], AF.Copy,
                                        scale=1.0 / D)                # c3
            # broadcast (c1,c2,c3) to all 128 partitions via rank-1 matmul
            i_bc = nc.tensor.matmul(PB[:, 0:3], ONESF[0:1, 0:D], PK[0:1, 0:3],
                                    start=True, stop=True)
            i_sb = nc.scalar.copy(SB[:, 0:3], PB[:, 0:3])

            i_exp = [None, None]
            i_x2 = [None, None]
            i_xb = [None, None]
            for h in range(2):
                # e = exp(1 - dc) -> bf16
                i_exp[h] = nc.scalar.activation(ET[h][:], DCT[h][:], AF.Exp,
                                                bias=OB, scale=-1.0)
                # f32 -> bf16 cast on DVE, then x2 = x*x -> bf16
                i_xb[h] = nc.vector.tensor_copy(XB[h][:], XT[h][:])
                i_x2[h] = nc.vector.tensor_mul(X2[h][:], XB[h][:], XB[h][:])
            # ACT is FIFO: the tiny scalar chain must not get stuck behind
            # the big exp passes.
            add_dep_helper(i_exp[0].ins, i_c2.ins, sync=False,
                           reason="scalar chain ahead of exps on ACT")
            add_dep_helper(i_exp[0].ins, i_c3.ins, sync=False,
                           reason="scalar chain ahead of exps on ACT")
            add_dep_helper(i_exp[0].ins, i_sb.ins, sync=False,
                           reason="SB copy ahead of exps on ACT")

            # scaled stationary slabs (first 128 rotated columns = own rows)
            i_xsc = nc.vector.tensor_scalar_mul(XSC[:], XB[0][:, 0:ROWS],
                                                SB[:, 1:2])
            i_esc = nc.vector.tensor_scalar_mul(ESC[:], ET[0][:, 0:ROWS],
                                                SB[:, 2:3])

            # r-terms become full K=128 matmuls straight from X2:
            #   c1*r_j broadcast down rows:  (c1*ones)^T @ X2[h]
            #   c1*r_i broadcast across cols: (c1*X2_slab)^T @ ones
            i_onem = nc.vector.tensor_scalar_mul(ONEM[:], ONESB[:, 0:ROWS],
                                                 SB[:, 0:1])
            i_x2sc = nc.vector.tensor_scalar_mul(X2SC[:], X2[0][:, 0:ROWS],
                                                 SB[:, 0:1])
            # DVE: everything PE group 0 needs must precede X2[1], which
            # stalls on the second cast-DMA's late completion.
            for dep in (i_sb, i_xsc, i_onem, i_x2sc, i_esc):
                add_dep_helper(i_x2[1].ins, dep.ins, sync=False,
                               reason="group-0 feeders ahead of X2[1] on DVE")

            i_recip = [None, None]
            i_mm1 = [None, None]
            i_mm2 = [None, None]
            i_mm3 = [None, None]
            i_mm4 = [None, None]
            for h in range(2):
                i_mm1[h] = nc.tensor.matmul(PS[h][:], XSC[:], XB[h][:],
                                            start=True, stop=False)
                # + c1*r_j (row term)
                i_mm3[h] = nc.tensor.matmul(PS[h][:], ONEM[:], X2[h][:],
                                            start=False, stop=False)
                # + c1*r_i (column term)
                i_mm4[h] = nc.tensor.matmul(PS[h][:], X2SC[:],
                                            ONESB[:, 0:H], start=False,
                                            stop=False)
                # E term last: ESC arrives latest (behind exp0 + broadcast)
                i_mm2[h] = nc.tensor.matmul(PS[h][:], ESC[:], ET[h][:],
                                            start=False, stop=True)
                # ~5x faster than reciprocal(), ~18 correct bits - far inside
                # tolerance; den is bounded away from 0 (>= a*min(e)^2 > 0)
                i_recip[h] = nc.vector.reciprocal_approx_fast(SIM[h][:],
                                                              PS[h][:])
            # PE program order: broadcast first, then group 0, then group 1
            pe_order = [i_bc, i_mm1[0], i_mm3[0], i_mm4[0], i_mm2[0],
                        i_mm1[1], i_mm3[1], i_mm4[1], i_mm2[1]]
            for a, b in zip(pe_order[1:], pe_order[:-1]):
                add_dep_helper(a.ins, b.ins, sync=False,
                               reason="PE program order")
            for dep in (i_sb, i_x2[0], i_x2[1], i_xsc, i_esc, i_onem,
                        i_x2sc):
                add_dep_helper(i_recip[0].ins, dep.ins, sync=False,
                               reason="keep DVE feeders ahead of recip0")

            # diagonal := 1.0 (local columns 0:128 hold the diagonal block)
            nc.gpsimd.affine_select(
                SIM[0][:, 0:ROWS], SIM[0][:, 0:ROWS],
                pattern=[[1, ROWS]], compare_op=mybir.AluOpType.not_equal,
                fill=1.0, base=0, channel_multiplier=-1,
            )

            # out0 on the ACT ring so out1's issue isn't queued behind it
            nc.scalar.dma_start(out[:, 0:H], SIM[0][:])
            nc.sync.dma_start(out[:, H:B], SIM[1][:])
    nc.finalize()
    return nc


def _prep_in_maps(x, dc, dc_param):
    x = np.ascontiguousarray(np.asarray(x, dtype=np.float32))
    dc = np.ascontiguousarray(np.asarray(dc, dtype=np.float32))
    a = np.asarray(dc_param, dtype=np.float32).reshape(1, 1)
    xT = np.ascontiguousarray(x.T)
    dcT = np.ascontiguousarray(dc.T)
    in_maps = []
    for c in range(NCORES):
        sh = c * ROWS
        in_maps.append({
            "xT": np.ascontiguousarray(np.roll(xT, -sh, axis=1)),
            "dcT": np.ascontiguousarray(np.roll(dcT, -sh, axis=1)),
            "apar": a,
        })
    return in_maps


def _unshard(results):
    out = np.empty((B, B), dtype=np.float32)
    for c in range(NCORES):
        sh = c * ROWS
        out[sh:sh + ROWS, :] = np.roll(results[c]["out"], sh, axis=1)
    return out


def kernel(x, dc, dc_param):
    nc = build_nc()
    res = run_bass_kernel_spmd(nc, _prep_in_maps(x, dc, dc_param),
                               list(range(NCORES)))
    return _unshard(res.results)


def _ensure_ntff_hook():
    """The agent image's ``antenv`` lacks ``axon_hooks``; synthesize it and
    register the ctypes NTFF-profiling hook so trace=True works."""
    import sys
    import types
    try:
        from antenv.axon_hooks import get_axon_ntff_profile_hook  # noqa: F401
        return
    except ImportError:
        pass
    mod = types.ModuleType("antenv.axon_hooks")
    mod._hook = None

    def set_axon_ntff_profile_hook(h):
        mod._hook = h

    def get_axon_ntff_profile_hook():
        return mod._hook

    mod.set_axon_ntff_profile_hook = set_axon_ntff_profile_hook
    mod.get_axon_ntff_profile_hook = get_axon_ntff_profile_hook
    sys.modules["antenv.axon_hooks"] = mod
    try:
        from trn_agent_boot.trn_boot import _ntff_profile_via_ctypes
        mod._hook = _ntff_profile_via_ctypes("/opt/axon/libaxon_pjrt.so")
    except Exception as e:  # degrade to no-trace
        print(f"ntff hook setup failed: {e}", file=sys.stderr)


def kernel_traced(x, dc, dc_param, reps=3):
    """Like kernel() but captures a neuron-profile trace; returns
    (output, best_exec_time_ns, trace_path). Runs `reps` times (the NEFF is
    compiled once and cached) and reports the fastest - exec time is noisy
    run-to-run (chip power state, co-tenants)."""
    _ensure_ntff_hook()
    nc = build_nc()
    in_maps = _prep_in_maps(x, dc, dc_param)
    best = None
    for _ in range(reps):
        res = run_bass_kernel_spmd(nc, in_maps, list(range(NCORES)),
                                   trace=True,
                                   trace_cores=list(range(NCORES)))
        print(f"  rep exec_time_ns: {res.exec_time_ns}")
        if best is None or (res.exec_time_ns or 1 << 60) < (
                best.exec_time_ns or 1 << 60):
            best = res
    trace_path = None
    if best.instructions_and_trace is not None:
        trace_path = best.instructions_and_trace[1]
    return _unshard(best.results), best.exec_time_ns, trace_path
